# revision 52
# baseline (speedup 1.0000x reference)
"""Trainium2 Bass kernel: dense-masked sliding-window attention.

nn.Module semantics (see harness reference):
    B,S,E,H,W = 1, 4096, 1024, 16, 256; D = 64
    qkv = x @ w_qkv -> q,k,v  [B,S,H,D]
    scores = q k^T / sqrt(D), masked to the sliding causal window
             (key j allowed for query i iff i-W < j <= i)
    out = softmax(scores) v  -> [B,S,E] @ w_out

Sharding: sequence-parallel over 8 NeuronCores. Core c owns queries
[512c, 512c+512) and receives a 256-row key/value halo on the left; no
collectives are needed (host concatenates the per-core output rows).

Per-core kernel layout (attention-value swap):
  - x is shipped pre-transposed ([E, 768] feature-major) so every matmul
    contracts over the partition dim without on-device transposes.
  - q^T, k^T are produced transposed ([dims, seq]); v natural
    ([seq, dims]) with a 1.0 column appended per head (65-wide heads).
    PSUM->SBUF copies are spread over ACT and DVE (GPSIMD cannot touch
    PSUM on TRN2).
  - scores are built transposed ([t, s]) in [128, 512] single-bank PSUM
    tiles; softmax skips the max-subtraction (scores are O(1): exp can't
    overflow); the window/causal mask is applied multiplicatively on
    exp(scores) with mask data taken from the allowed_mask input.
  - attention-value products run with exp(scores) STATIONARY and v
    MOVING, so the output lands [q, 65] per (head, query-block) with
    queries on partitions and the softmax denominator in column 64 (from
    the vt ones column): normalization is one per-partition reciprocal +
    one strided multiply per head.  A PE transpose (identity matmul)
    restores [feat, q] for the output projection.
  - emission order interleaves k-proj/QK/v-proj so the PE stays fed
    while the exp chain drains on ACT; a junk-matmul warmup ramps the PE
    p-state under the initial DMA wait (cold start only).
  - all matmuls run in fp16 (full PE rate).  End-to-end error vs the
    fp32 reference is ~4e-4 scale-relative.
"""

import numpy as np
from contextlib import ExitStack

import concourse.bass as bass
import concourse.tile as tile
from concourse import bacc, mybir
from concourse.bass_utils import run_bass_kernel_spmd

F32 = mybir.dt.float32
F16 = mybir.dt.float16

B, S, E, H, W = 1, 4096, 1024, 16, 256
D = E // H  # 64
SCALE = D ** -0.5
N_CORES = 8
SQ = S // N_CORES          # 512 queries per core
HALO = W                   # 256 halo keys
SK = SQ + HALO             # 768 key rows per core
KC = E // 128              # 8 contraction chunks
QB = SQ // 128             # 4 query blocks per core
TC = SK // 128             # 6 key chunks per core

# Attention band units (T = key chunk, qb = query block), packed into
# three 512-column groups so each score tile is one PSUM bank.  Within a
# group, units sharing a T are contiguous in qb so QK needs one matmul
# per run.  col = offset in the per-head 1536-wide score/mask space.
UNITS = [
    (2, 0, 0), (2, 1, 128), (2, 2, 256), (0, 0, 384),          # block 0
    (1, 0, 512), (1, 1, 640), (4, 2, 768), (4, 3, 896),        # block 1
    (3, 1, 1024), (3, 2, 1152), (3, 3, 1280), (5, 3, 1408),    # block 2
]
# QK matmul runs per block: (T, qb0, nqb, local col0)
QK_RUNS = [
    [(2, 0, 3, 0), (0, 0, 1, 384)],
    [(1, 0, 2, 0), (4, 2, 2, 256)],
    [(3, 1, 3, 0), (5, 3, 1, 384)],
]
NSLICE_COLS = 1536


def _emit_body(ctx: ExitStack, tc_: "tile.TileContext", xT_d, wq_d, wk_d, wv_d,
               wout_d, pmask_d, ident_d, out_d, warm=True):
    nc = tc_.nc
    P = 128

    xt_pool = ctx.enter_context(tc_.tile_pool(name="xt", bufs=KC))
    # 16-deep: wq+wk stay resident through the interleaved k/QK/v phase;
    # wv reuses wq's early-freed slots, wo reuses wk's (a 10-deep ring
    # deadlocks: wv DMAs would wait on wk slots whose last consumer is
    # behind the v-proj matmuls in the PE queue).
    w_pool = ctx.enter_context(tc_.tile_pool(name="w", bufs=16))
    qt_pool = ctx.enter_context(tc_.tile_pool(name="qt", bufs=KC))
    kt_pool = ctx.enter_context(tc_.tile_pool(name="kt", bufs=KC))
    v_pool = ctx.enter_context(tc_.tile_pool(name="v", bufs=TC))
    pm_pool = ctx.enter_context(tc_.tile_pool(name="pm", bufs=1))
    id_pool = ctx.enter_context(tc_.tile_pool(name="id", bufs=1))
    et_pool = ctx.enter_context(tc_.tile_pool(name="et", bufs=KC))
    at_pool = ctx.enter_context(tc_.tile_pool(name="at", bufs=1))
    atT_pool = ctx.enter_context(tc_.tile_pool(name="atT", bufs=KC))
    rc_pool = ctx.enter_context(tc_.tile_pool(name="rc", bufs=4))
    os_pool = ctx.enter_context(tc_.tile_pool(name="os", bufs=2))
    one_pool = ctx.enter_context(tc_.tile_pool(name="one", bufs=1))
    # PSUM: every working tile is one bank (score/proj/avout/transpose
    # all share the 8-deep "sc" ring).
    sc_pool = ctx.enter_context(tc_.tile_pool(name="sc", bufs=8, space="PSUM"))

    # ---- PE warmup: junk matmuls ramp the PE p-state while the first
    # DMAs land.  Only worth it on a cold start: in a timing loop the PE
    # stays hot across iterations and the junk would add ~3us/iter.
    if warm:
        junk = one_pool.tile([P, 256], F16, tag="junk")
        nc.vector.memset(junk[:], 0.0)
        jp = sc_pool.tile([P, 256], F32, tag="sc", name="warm")
        for _ in range(14):
            nc.tensor.matmul(jp[:], junk[:, 0:128], junk[:],
                             start=True, stop=True)

    # ---- loads (wq/xt first: they gate the first matmuls) ----------------
    wq, xt = [], []
    for k in range(KC):
        t = w_pool.tile([P, 1024], F16, tag="w", bufs=16)
        nc.sync.dma_start(t[:], wq_d.ap()[k * P:(k + 1) * P, :])
        wq.append(t)
        t = xt_pool.tile([P, SK], F16, tag="xt")
        nc.sync.dma_start(t[:], xT_d.ap()[k * P:(k + 1) * P, :])
        xt.append(t)

    # ---- q^T [E, SQ]: stationary = w_q chunk columns, moving = x^T -------
    qt = []
    for n in range(KC):
        ps = sc_pool.tile([P, 512], F32, tag="sc", name=f"qtp{n}")
        for k in range(KC):
            nc.tensor.matmul(ps[:], wq[k][:, n * P:(n + 1) * P],
                             xt[k][:, HALO:SK], start=(k == 0), stop=(k == KC - 1))
        t = qt_pool.tile([P, SQ], F16, tag="qt")
        nc.scalar.copy(t[:], ps[:])
        qt.append(t)

    # ---- k^T [E, SK] interleaved with attention scores -------------------
    # QK for pair p is emitted after k-proj chunk p+1 so the PE never waits
    # on the copy of kt[p]; exp+mask trail on ACT/DVE.
    wk = []
    for k in range(KC):
        t = w_pool.tile([P, 1024], F16, tag="w", bufs=16)
        nc.sync.dma_start(t[:], wk_d.ap()[k * P:(k + 1) * P, :])
        wk.append(t)
    pm = pm_pool.tile([P, NSLICE_COLS], F16)
    nc.sync.dma_start(pm[:], pmask_d.ap()[:])
    ident = id_pool.tile([P, P], F16)
    nc.sync.dma_start(ident[:], ident_d.ap()[:])
    ones_f = one_pool.tile([P, 1], F32, tag="ones")
    nc.vector.memset(ones_f[:], 1.0)

    kt = []
    et = []

    def emit_kchunk(n):
        psa = sc_pool.tile([P, 512], F32, tag="sc", name=f"ktpa{n}")
        psb = sc_pool.tile([P, 512], F32, tag="sc", name=f"ktpb{n}")
        for k in range(KC):
            nc.tensor.matmul(psa[:], wk[k][:, n * P:(n + 1) * P],
                             xt[k][:, 0:512], start=(k == 0), stop=(k == KC - 1))
            nc.tensor.matmul(psb[:, 0:256], wk[k][:, n * P:(n + 1) * P],
                             xt[k][:, 512:768], start=(k == 0), stop=(k == KC - 1))
        t = kt_pool.tile([P, SK], F16, tag="kt")
        nc.scalar.copy(t[:, 0:512], psa[:])
        nc.vector.tensor_copy(t[:, 512:768], psb[:, 0:256])
        kt.append(t)

    def emit_qk(p):
        # et[p] holds exp(scores) for both heads of pair p: [128, 2, 1536].
        e = et_pool.tile([P, 2 * NSLICE_COLS], F16, tag="et")
        ev = e[:].rearrange("p (s c) -> p s c", s=2)
        for blk in range(3):
            sps = []
            for sub in range(2):
                r0 = 64 * sub
                sp = sc_pool.tile([P, 512], F32, tag="sc")
                sps.append(sp)
                for (T, q0, nq, c0) in QK_RUNS[blk]:
                    nc.tensor.matmul(
                        sp[:, c0:c0 + nq * 128],
                        kt[p][r0:r0 + 64, T * P:(T + 1) * P],
                        qt[p][r0:r0 + 64, q0 * 128:(q0 + nq) * 128],
                        start=True, stop=True, tile_position=(r0, 0))
            for sub in range(2):
                nc.scalar.activation(ev[:, sub, blk * 512:(blk + 1) * 512],
                                     sps[sub][:],
                                     mybir.ActivationFunctionType.Exp)
        nc.vector.tensor_tensor(
            ev[:, :, :], ev[:, :, :],
            pm[:, None, :].broadcast_to([P, 2, NSLICE_COLS]),
            mybir.AluOpType.mult)
        et.append(e)

    # ---- v natural [SK, E] (emitted interleaved below) -------------------
    wv = []
    for k in range(KC):
        t = w_pool.tile([P, 1024], F16, tag="w", bufs=16)
        nc.sync.dma_start(t[:], wv_d.ap()[k * P:(k + 1) * P, :])
        wv.append(t)
    vt = []

    def emit_vchunk(sc):
        # vt rows are [v(64) | 1.0] per head (65 cols): the AV matmul then
        # emits values AND the softmax denominator in one go.
        psa = sc_pool.tile([P, 512], F32, tag="sc", name=f"vpa{sc}")
        psb = sc_pool.tile([P, 512], F32, tag="sc", name=f"vpb{sc}")
        for k in range(KC):
            nc.tensor.matmul(psa[:], xt[k][:, sc * P:(sc + 1) * P],
                             wv[k][:, 0:512], start=(k == 0), stop=(k == KC - 1))
            nc.tensor.matmul(psb[:], xt[k][:, sc * P:(sc + 1) * P],
                             wv[k][:, 512:1024], start=(k == 0), stop=(k == KC - 1))
        t = v_pool.tile([P, H * 65], F16, tag="v")
        tv = t[:].rearrange("p (h c) -> p h c", h=H)
        nc.vector.tensor_copy(
            tv[:, 0:8, 0:64], psa[:].rearrange("p (h c) -> p h c", h=8))
        nc.vector.tensor_copy(
            tv[:, 8:16, 0:64], psb[:].rearrange("p (h c) -> p h c", h=8))
        nc.gpsimd.tensor_copy(
            tv[:, :, 64:65], ones_f[:, None, :].broadcast_to([P, H, 1]))
        vt.append(t)

    # PE order: k-chunks lead their pair's QK by one so the copy of kt[p]
    # is never on the critical path; two v-chunks slot in early to keep
    # the exp chain fed, the remaining four run AFTER the last QK so the
    # exp+mask pipeline fully drains (on ACT/DVE) before the AV phase --
    # otherwise the in-order DVE queue serializes mask(7) ahead of the
    # recip/norm chain and stalls the PE.
    emit_kchunk(0)
    for n in range(1, KC):
        emit_kchunk(n)
        emit_qk(n - 1)
        if n >= 2:
            emit_vchunk(n - 2)
    emit_qk(KC - 1)

    wo = []
    for k in range(KC):
        t = w_pool.tile([P, 1024], F16, tag="w", bufs=16)
        nc.sync.dma_start(t[:], wout_d.ap()[k * P:(k + 1) * P, :])
        wo.append(t)

    # ---- attention values + denominators + normalize ---------------------
    # avout per head: [q, (qb, 65)] with queries on partitions; col 64 of
    # each 65-group is the softmax denominator (from the vt ones column).
    # at layout [q, (pair, qb, sub, d)]: the PE transpose needs each
    # (pair, qb) feature block contiguous (matmul weights APs must have a
    # single free dimension).
    at = at_pool.tile([P, 4096], F16)
    units_by_qb = [[u for u in UNITS if u[1] == qb] for qb in range(QB)]
    atT = []

    def emit_transpose(p):
        psT = sc_pool.tile([P, SQ], F16, tag="sc", name=f"tr{p}")
        for qb in range(QB):
            nc.tensor.transpose(psT[:, qb * P:(qb + 1) * P],
                                at[:, p * 512 + qb * P:p * 512 + (qb + 1) * P],
                                ident[:])
        t = atT_pool.tile([P, SQ], F16, tag="atT")
        if p % 2 == 0:
            nc.scalar.copy(t[:], psT[:])
        else:
            nc.vector.tensor_copy(t[:], psT[:])
        atT.append(t)

    for p in range(KC):
        ev = et[p][:].rearrange("p (s c) -> p s c", s=2)
        avs = []
        for sub in range(2):
            h = 2 * p + sub
            av = sc_pool.tile([P, QB * 65], F32, tag="sc", name=f"av{h}")
            avs.append(av)
            for qb in range(QB):
                us = units_by_qb[qb]
                for i, (T, _, c0) in enumerate(us):
                    nc.tensor.matmul(
                        av[:, qb * 65:qb * 65 + 65],
                        ev[:, sub, c0:c0 + 128],
                        vt[T][:, h * 65:(h + 1) * 65],
                        start=(i == 0), stop=(i == len(us) - 1))
        rc = rc_pool.tile([P, 8], F32, tag="rc")
        at_v = at[:].rearrange("p (pp q s d) -> p pp q s d", pp=KC, q=QB, s=2)
        for sub in range(2):
            nc.vector.reciprocal(
                rc[:, sub * 4:(sub + 1) * 4],
                avs[sub][:].rearrange("p (q c) -> p q c", c=65)[:, :, 64])
        for sub in range(2):
            nc.vector.tensor_tensor(
                at_v[:, p, :, sub, :],
                avs[sub][:].rearrange("p (q c) -> p q c", c=65)[:, :, 0:D],
                rc[:, sub * 4:(sub + 1) * 4][:, :, None]
                    .broadcast_to([P, QB, D]),
                mybir.AluOpType.mult)
    # transpose at [q, f] -> atT [f, q] via PE identity matmuls, batched
    # after the AV loop: the norm chain on DVE drains while the PE runs
    # the remaining AVs, so the transposes rarely wait.
    for p in range(KC):
        emit_transpose(p)

    # ---- output projection ----------------------------------------------
    # Blocks 0..2: one copy per half (ACT||DVE) + one DMA per half (the
    # DMAs overlap later matmuls).  Last block: 256-col column groups so
    # the copy+DMA of each group hides under the next group's matmuls and
    # the serial tail is just the final 256 columns.
    for sb in range(QB - 1):
        psa = sc_pool.tile([P, 512], F32, tag="sc", name=f"opa{sb}")
        psb = sc_pool.tile([P, 512], F32, tag="sc", name=f"opb{sb}")
        for c in range(KC):
            nc.tensor.matmul(psa[:], atT[c][:, sb * P:(sb + 1) * P],
                             wo[c][:, 0:512], start=(c == 0), stop=(c == KC - 1))
            nc.tensor.matmul(psb[:], atT[c][:, sb * P:(sb + 1) * P],
                             wo[c][:, 512:1024], start=(c == 0), stop=(c == KC - 1))
        ob = os_pool.tile([P, E], F16, tag="os")
        nc.scalar.copy(ob[:, 0:512], psa[:])
        nc.vector.tensor_copy(ob[:, 512:1024], psb[:])
        nc.sync.dma_start(out_d.ap()[sb * P:(sb + 1) * P, 0:512],
                          ob[:, 0:512])
        nc.sync.dma_start(out_d.ap()[sb * P:(sb + 1) * P, 512:1024],
                          ob[:, 512:1024])
    # Last block: cols 0:768 first, then 256 cols alone so the final
    # serial copy+DMA tail is only a quarter block.
    sb = QB - 1
    psa = sc_pool.tile([P, 512], F32, tag="sc", name="opa3")
    psb = sc_pool.tile([P, 512], F32, tag="sc", name="opb3")
    ob = os_pool.tile([P, E], F16, tag="os")
    for c in range(KC):
        nc.tensor.matmul(psa[:], atT[c][:, sb * P:(sb + 1) * P],
                         wo[c][:, 0:512], start=(c == 0), stop=(c == KC - 1))
        nc.tensor.matmul(psb[:, 0:256], atT[c][:, sb * P:(sb + 1) * P],
                         wo[c][:, 512:768], start=(c == 0), stop=(c == KC - 1))
    nc.scalar.copy(ob[:, 0:512], psa[:])
    nc.vector.tensor_copy(ob[:, 512:768], psb[:, 0:256])
    nc.sync.dma_start(out_d.ap()[sb * P:(sb + 1) * P, 0:512], ob[:, 0:512])
    nc.sync.dma_start(out_d.ap()[sb * P:(sb + 1) * P, 512:768],
                      ob[:, 512:768])
    for c in range(KC):
        nc.tensor.matmul(psb[:, 256:512], atT[c][:, sb * P:(sb + 1) * P],
                         wo[c][:, 768:1024], start=(c == 0), stop=(c == KC - 1))
    nc.vector.tensor_copy(ob[:, 768:1024], psb[:, 256:512])
    nc.sync.dma_start(out_d.ap()[sb * P:(sb + 1) * P, 768:1024],
                      ob[:, 768:1024])


def build(n_iters: int = 1):
    nc = bacc.Bacc("TRN2", target_bir_lowering=False, debug=False,
                   num_devices=N_CORES)
    xT_d = nc.dram_tensor("xT", [E, SK], F16, kind="ExternalInput")
    wq_d = nc.dram_tensor("wq", [E, E], F16, kind="ExternalInput")
    wk_d = nc.dram_tensor("wk", [E, E], F16, kind="ExternalInput")
    wv_d = nc.dram_tensor("wv", [E, E], F16, kind="ExternalInput")
    wout_d = nc.dram_tensor("wout", [E, E], F16, kind="ExternalInput")
    pmask_d = nc.dram_tensor("pmask", [128, NSLICE_COLS], F16,
                             kind="ExternalInput")
    ident_d = nc.dram_tensor("ident", [128, 128], F16, kind="ExternalInput")
    out_d = nc.dram_tensor("out", [SQ, E], F16, kind="ExternalOutput")
    with tile.TileContext(nc) as tc_, ExitStack() as ctx:
        if n_iters > 1:
            with tc_.For_i(0, n_iters, 1):
                _emit_body(ctx, tc_, xT_d, wq_d, wk_d, wv_d, wout_d, pmask_d,
                           ident_d, out_d, warm=False)
        else:
            _emit_body(ctx, tc_, xT_d, wq_d, wk_d, wv_d, wout_d, pmask_d,
                       ident_d, out_d, warm=True)
    nc.compile()
    return nc


def make_in_maps(x, allowed_mask, w_qkv, w_out):
    """Shard the full inputs into per-core input maps (host marshaling)."""
    x2 = np.asarray(x, dtype=np.float32).reshape(S, E)
    wqkv = np.asarray(w_qkv, dtype=np.float32)
    wq = np.ascontiguousarray(wqkv[:, 0:E]) * np.float32(SCALE)
    wk = np.ascontiguousarray(wqkv[:, E:2 * E])
    wv = np.ascontiguousarray(wqkv[:, 2 * E:3 * E])
    wout = np.ascontiguousarray(np.asarray(w_out, dtype=np.float32))
    am = np.asarray(allowed_mask).reshape(S, S)
    ident = np.eye(128, dtype=np.float16)

    xT = np.ascontiguousarray(x2.T)  # [E, S]
    in_maps = []
    for c in range(N_CORES):
        lo = c * SQ - HALO
        xTc = np.zeros((E, SK), dtype=np.float32)
        ofs = max(0, -lo)
        xTc[:, ofs:] = xT[:, lo + ofs:c * SQ + SQ]
        pmask = np.zeros((128, NSLICE_COLS), dtype=np.float32)
        for (T, qb, col) in UNITS:
            t0 = lo + T * 128
            if t0 + 128 <= 0:
                continue
            tlo = max(0, -t0)
            s0 = c * SQ + qb * 128
            blk = am[s0:s0 + 128, t0 + tlo:t0 + 128]  # [s, t]
            pmask[tlo:128, col:col + 128] = blk.T.astype(np.float32)
        in_maps.append({
            "xT": xTc.astype(np.float16),
            "wq": wq.astype(np.float16),
            "wk": wk.astype(np.float16),
            "wv": wv.astype(np.float16),
            "wout": wout.astype(np.float16),
            "pmask": pmask.astype(np.float16),
            "ident": ident,
        })
    return in_maps


_CACHED_NC = None


def kernel(x, allowed_mask, w_qkv, w_out):
    global _CACHED_NC
    if _CACHED_NC is None:
        _CACHED_NC = build()
    in_maps = make_in_maps(x, allowed_mask, w_qkv, w_out)
    res = run_bass_kernel_spmd(_CACHED_NC, in_maps, list(range(N_CORES)))
    out = np.concatenate([res.results[c]["out"].astype(np.float32)
                          for c in range(N_CORES)], axis=0)
    return out.reshape(B, S, E)
